# revision 1
# baseline (speedup 1.0000x reference)
"""Trainium2 Bass kernel for PhaseCoherenceComputer.

coherence[b,h,q,k] = mean_d cos(phases_q[b,h,q,d] - phases_k[b,h,k,d])
                   = (cos_q @ cos_k^T + sin_q @ sin_k^T) / 64

Shapes: phases_q/k [2, 8, 2048, 64] f32 -> out [2, 8, 2048, 2048] f32.

Strategy (8 NeuronCores, data-parallel over the 16 (b,h) pairs, 2 per core):
- Host: for each pair, transpose phases to [64, 2048] (harmonic d on
  partitions) and stack the *cos argument* (x + pi/2, range-reduced to
  [-pi, pi]) on partitions 0:64 above the *sin argument* (x range-reduced)
  on partitions 64:128. Range reduction happens on host because the ACT
  Sin spline is only accurate on [-pi, pi].
- Device: one Sin activation turns each [128, 2048] block into
  U = [cos_q^T; sin_q^T] (resp. V for k), written as float32r so the
  tensor engine runs at full rate (fp32 matmuls are 1/4 rate; float32r
  is full rate for moving dim >= 256 with ~1e-4 relative rounding).
- One K=128 matmul per [128 q x 512 k] output tile computes
  cos_q cos_k + sin_q sin_k in a single pass (cos/sin concatenated along
  the contraction dim). PSUM accumulates fp32; evacuation applies the
  1/64 scale, alternating VectorE/ScalarE; DMA out [128, 2048] blocks.
"""

import sys

import numpy as np

try:
    import concourse.bacc as bacc
except ImportError:  # fresh interpreter without the axon site path
    for _p in ("/opt/trn_rl_repo", "/root/.axon_site/_ro/trn_rl_repo"):
        if _p not in sys.path:
            sys.path.insert(0, _p)
    import concourse.bacc as bacc

import concourse.mybir as mybir
import concourse.tile as tile
from concourse.bass_utils import run_bass_kernel_spmd

F32 = mybir.dt.float32
F32R = mybir.dt.float32r

B, H, S, D = 2, 8, 2048, 64
N_CORES = 8
PAIRS_PER_CORE = (B * H) // N_CORES  # 2
Q_TILE = 128  # output rows per matmul (PSUM partitions)
K_TILE = 512  # output cols per matmul (one PSUM bank)
N_QT = S // Q_TILE  # 16
N_KT = S // K_TILE  # 4

_NC_CACHE = {}


def build_kernel():
    """Per-core SPMD program. Inputs qc/kc [PAIRS, 128, S]: partitions 0:64
    carry the cos argument, 64:128 the sin argument, both in [-pi, pi]."""
    nc = bacc.Bacc("TRN2", target_bir_lowering=False, debug=False)
    qc = nc.dram_tensor("qc", [PAIRS_PER_CORE, 128, S], F32, kind="ExternalInput")
    kc = nc.dram_tensor("kc", [PAIRS_PER_CORE, 128, S], F32, kind="ExternalInput")
    out = nc.dram_tensor("out", [PAIRS_PER_CORE, S, S], F32, kind="ExternalOutput")

    with tile.TileContext(nc) as tc:
        with (
            tc.tile_pool(name="raw", bufs=2) as rawpool,
            tc.tile_pool(name="uv", bufs=2) as uvpool,
            tc.tile_pool(name="ot", bufs=6) as opool,
            tc.tile_pool(name="psum", bufs=2, space="PSUM") as ppool,
        ):
            for p in range(PAIRS_PER_CORE):
                qraw = rawpool.tile([128, S], F32, tag="raw")
                nc.sync.dma_start(out=qraw[:], in_=qc[p])
                u = uvpool.tile([128, S], F32R, tag="u")
                nc.scalar.activation(u[:], qraw[:], mybir.ActivationFunctionType.Sin)

                kraw = rawpool.tile([128, S], F32, tag="raw")
                nc.sync.dma_start(out=kraw[:], in_=kc[p])
                v = uvpool.tile([128, S], F32R, tag="v")
                nc.scalar.activation(v[:], kraw[:], mybir.ActivationFunctionType.Sin)

                for q in range(N_QT):
                    ps = ppool.tile([128, N_KT * K_TILE], F32, tag="ps")
                    for k in range(N_KT):
                        nc.tensor.matmul(
                            ps[:, k * K_TILE : (k + 1) * K_TILE],
                            u[:, q * Q_TILE : (q + 1) * Q_TILE],
                            v[:, k * K_TILE : (k + 1) * K_TILE],
                            start=True,
                            stop=True,
                        )
                    ot = opool.tile([128, S], F32, tag="ot")
                    if q % 2 == 0:
                        nc.vector.tensor_scalar_mul(ot[:], ps[:], 1.0 / D)
                    else:
                        nc.scalar.mul(ot[:], ps[:], 1.0 / D)
                    nc.sync.dma_start(
                        out=out[p, q * Q_TILE : (q + 1) * Q_TILE, :], in_=ot[:]
                    )
    nc.compile()
    return nc


def _prep(ph):
    """[16, S, D] phases -> [16, 128, S] stacked cos/sin arguments."""
    pht = ph.astype(np.float64).transpose(0, 2, 1)  # [16, D, S]
    cos_arg = np.mod(pht + (np.pi / 2 + np.pi), 2 * np.pi) - np.pi
    sin_arg = np.mod(pht + np.pi, 2 * np.pi) - np.pi
    return np.concatenate([cos_arg, sin_arg], axis=1).astype(np.float32)


def kernel(phases_q, phases_k, _trace=False):
    pq = np.asarray(phases_q, dtype=np.float32).reshape(B * H, S, D)
    pk = np.asarray(phases_k, dtype=np.float32).reshape(B * H, S, D)
    qc = _prep(pq)  # [16, 128, S]
    kc = _prep(pk)

    in_maps = []
    for c in range(N_CORES):
        sl = slice(c * PAIRS_PER_CORE, (c + 1) * PAIRS_PER_CORE)
        in_maps.append(
            {"qc": np.ascontiguousarray(qc[sl]), "kc": np.ascontiguousarray(kc[sl])}
        )

    if "nc" not in _NC_CACHE:
        _NC_CACHE["nc"] = build_kernel()
    nc = _NC_CACHE["nc"]

    res = run_bass_kernel_spmd(
        nc, in_maps, core_ids=list(range(N_CORES)), trace=_trace
    )
    full = np.concatenate([r["out"] for r in res.results], axis=0)
    out = full.reshape(B, H, S, S)
    if _trace:
        return out, res
    return out


# revision 2
# speedup vs baseline: 1.0171x; 1.0171x over previous
"""Trainium2 Bass kernel for PhaseCoherenceComputer.

coherence[b,h,q,k] = mean_d cos(phases_q[b,h,q,d] - phases_k[b,h,k,d])
                   = (cos_q @ cos_k^T + sin_q @ sin_k^T) / 64

Shapes: phases_q/k [2, 8, 2048, 64] f32 -> out [2, 8, 2048, 2048] f32.

Strategy (8 NeuronCores, data-parallel over the 16 (b,h) pairs, 2 per core):
- Host: for each pair, transpose phases to [64, 2048] (harmonic d on
  partitions) and stack the *cos argument* (x + pi/2, range-reduced to
  [-pi, pi]) on partitions 0:64 above the *sin argument* (x range-reduced)
  on partitions 64:128. Range reduction happens on host because the ACT
  Sin spline is only accurate on [-pi, pi].
- Device: one Sin activation turns each [128, 2048] block into
  U = [cos_q^T; sin_q^T] (resp. V for k), written as float32r so the
  tensor engine runs at full rate (fp32 matmuls are 1/4 rate; float32r
  is full rate for moving dim >= 256 with ~1e-4 relative rounding).
- One K=128 matmul per [128 q x 512 k] output tile computes
  cos_q cos_k + sin_q sin_k in a single pass (cos/sin concatenated along
  the contraction dim). PSUM accumulates fp32; evacuation applies the
  1/64 scale, alternating VectorE/ScalarE; DMA out [128, 2048] blocks.
"""

import sys

import numpy as np

try:
    import concourse.bacc as bacc
except ImportError:  # fresh interpreter without the axon site path
    for _p in ("/opt/trn_rl_repo", "/root/.axon_site/_ro/trn_rl_repo"):
        if _p not in sys.path:
            sys.path.insert(0, _p)
    import concourse.bacc as bacc

import concourse.mybir as mybir
import concourse.tile as tile
from concourse.bass_utils import run_bass_kernel_spmd

F32 = mybir.dt.float32
F32R = mybir.dt.float32r

B, H, S, D = 2, 8, 2048, 64
N_CORES = 8
PAIRS_PER_CORE = (B * H) // N_CORES  # 2
Q_TILE = 128  # output rows per matmul (PSUM partitions)
K_TILE = 512  # output cols per matmul (one PSUM bank)
N_QT = S // Q_TILE  # 16
N_KT = S // K_TILE  # 4

_NC_CACHE = {}


def build_kernel():
    """Per-core SPMD program. Inputs qc/kc [PAIRS, 128, S]: partitions 0:64
    carry the cos argument, 64:128 the sin argument, both in [-pi, pi]."""
    nc = bacc.Bacc("TRN2", target_bir_lowering=False, debug=False)
    qc = nc.dram_tensor("qc", [PAIRS_PER_CORE, 128, S], F32, kind="ExternalInput")
    kc = nc.dram_tensor("kc", [PAIRS_PER_CORE, 128, S], F32, kind="ExternalInput")
    out = nc.dram_tensor("out", [PAIRS_PER_CORE, S, S], F32, kind="ExternalOutput")

    HC = S // 2  # half-row chunk for input DMA / sin / evac / out DMA

    with tile.TileContext(nc) as tc:
        with (
            tc.tile_pool(name="raw", bufs=2) as rawpool,
            tc.tile_pool(name="uv", bufs=2) as uvpool,
            tc.tile_pool(name="ot", bufs=8) as opool,
            tc.tile_pool(name="psum", bufs=2, space="PSUM") as ppool,
        ):
            for p in range(PAIRS_PER_CORE):
                # Input DMAs ride the gpsimd SWDGE queue so they never sit
                # ahead of output DMAs in the SP/ACT hardware queues.
                qraw = rawpool.tile([128, S], F32, tag="qraw")
                kraw = rawpool.tile([128, S], F32, tag="kraw")
                u = uvpool.tile([128, S], F32R, tag="u")
                v = uvpool.tile([128, S], F32R, tag="v")
                for h in range(2):
                    hs = slice(h * HC, (h + 1) * HC)
                    nc.gpsimd.dma_start(out=kraw[:, hs], in_=kc[p, :, hs])
                    nc.gpsimd.dma_start(out=qraw[:, hs], in_=qc[p, :, hs])
                    nc.scalar.activation(
                        v[:, hs], kraw[:, hs], mybir.ActivationFunctionType.Sin
                    )
                    nc.scalar.activation(
                        u[:, hs], qraw[:, hs], mybir.ActivationFunctionType.Sin
                    )

                for q in range(N_QT):
                    ps = ppool.tile([128, N_KT * K_TILE], F32, tag="ps")
                    for k in range(N_KT):
                        nc.tensor.matmul(
                            ps[:, k * K_TILE : (k + 1) * K_TILE],
                            u[:, q * Q_TILE : (q + 1) * Q_TILE],
                            v[:, k * K_TILE : (k + 1) * K_TILE],
                            start=True,
                            stop=True,
                        )
                    ot = opool.tile([128, S], F32, tag="ot")
                    for j in range(2):
                        hs = slice(j * HC, (j + 1) * HC)
                        # Evac PSUM->SBUF with the 1/64 scale, alternating
                        # engines; the out-DMA goes crosswise so each HWDGE
                        # queue (SP / ACT) carries half the output traffic.
                        if (2 * q + j) % 2 == 0:
                            nc.vector.tensor_scalar_mul(ot[:, hs], ps[:, hs], 1.0 / D)
                            nc.scalar.dma_start(
                                out=out[p, q * Q_TILE : (q + 1) * Q_TILE, hs],
                                in_=ot[:, hs],
                            )
                        else:
                            nc.scalar.mul(ot[:, hs], ps[:, hs], 1.0 / D)
                            nc.sync.dma_start(
                                out=out[p, q * Q_TILE : (q + 1) * Q_TILE, hs],
                                in_=ot[:, hs],
                            )
    nc.compile()
    return nc


def _prep(ph):
    """[16, S, D] phases -> [16, 128, S] stacked cos/sin arguments."""
    pht = ph.astype(np.float64).transpose(0, 2, 1)  # [16, D, S]
    cos_arg = np.mod(pht + (np.pi / 2 + np.pi), 2 * np.pi) - np.pi
    sin_arg = np.mod(pht + np.pi, 2 * np.pi) - np.pi
    return np.concatenate([cos_arg, sin_arg], axis=1).astype(np.float32)


def kernel(phases_q, phases_k, _trace=False):
    pq = np.asarray(phases_q, dtype=np.float32).reshape(B * H, S, D)
    pk = np.asarray(phases_k, dtype=np.float32).reshape(B * H, S, D)
    qc = _prep(pq)  # [16, 128, S]
    kc = _prep(pk)

    in_maps = []
    for c in range(N_CORES):
        sl = slice(c * PAIRS_PER_CORE, (c + 1) * PAIRS_PER_CORE)
        in_maps.append(
            {"qc": np.ascontiguousarray(qc[sl]), "kc": np.ascontiguousarray(kc[sl])}
        )

    if "nc" not in _NC_CACHE:
        _NC_CACHE["nc"] = build_kernel()
    nc = _NC_CACHE["nc"]

    res = run_bass_kernel_spmd(
        nc, in_maps, core_ids=list(range(N_CORES)), trace=_trace
    )
    full = np.concatenate([r["out"] for r in res.results], axis=0)
    out = full.reshape(B, H, S, S)
    if _trace:
        return out, res
    return out


# revision 3
# speedup vs baseline: 1.1108x; 1.0921x over previous
"""Trainium2 Bass kernel for PhaseCoherenceComputer.

coherence[b,h,q,k] = mean_d cos(phases_q[b,h,q,d] - phases_k[b,h,k,d])
                   = (cos_q @ cos_k^T + sin_q @ sin_k^T) / 64

Shapes: phases_q/k [2, 8, 2048, 64] f32 -> out [2, 8, 2048, 2048] f32.

Strategy (8 NeuronCores, data-parallel over the 16 (b,h) pairs, 2 per core):
- Host: for each pair, transpose phases to [64, 2048] (harmonic d on
  partitions) and stack the *cos argument* (x + pi/2, range-reduced to
  [-pi, pi]) on partitions 0:64 above the *sin argument* (x range-reduced)
  on partitions 64:128. Range reduction happens on host because the ACT
  Sin spline is only accurate on [-pi, pi].
- Device: one Sin activation turns each [128, 2048] block into
  U = [cos_q^T; sin_q^T] (resp. V for k), written as float32r so the
  tensor engine runs at full rate (fp32 matmuls are 1/4 rate; float32r
  is full rate for moving dim >= 256 with ~1e-4 relative rounding).
- One K=128 matmul per [128 q x 512 k] output tile computes
  cos_q cos_k + sin_q sin_k in a single pass (cos/sin concatenated along
  the contraction dim). PSUM accumulates fp32; evacuation applies the
  1/64 scale, alternating VectorE/ScalarE; DMA out [128, 2048] blocks.
"""

import sys

import numpy as np

try:
    import concourse.bacc as bacc
except ImportError:  # fresh interpreter without the axon site path
    for _p in ("/opt/trn_rl_repo", "/root/.axon_site/_ro/trn_rl_repo"):
        if _p not in sys.path:
            sys.path.insert(0, _p)
    import concourse.bacc as bacc

import concourse.mybir as mybir
import concourse.tile as tile
from concourse.bass_utils import run_bass_kernel_spmd

F32 = mybir.dt.float32
F32R = mybir.dt.float32r

B, H, S, D = 2, 8, 2048, 64
N_CORES = 8
PAIRS_PER_CORE = (B * H) // N_CORES  # 2
Q_TILE = 128  # output rows per matmul (PSUM partitions)
K_TILE = 512  # output cols per matmul (one PSUM bank)
N_QT = S // Q_TILE  # 16
N_KT = S // K_TILE  # 4

_NC_CACHE = {}


def build_kernel():
    """Per-core SPMD program. Inputs qc/kc [PAIRS, 128, S]: partitions 0:64
    carry the cos argument, 64:128 the sin argument, both in [-pi, pi]."""
    nc = bacc.Bacc("TRN2", target_bir_lowering=False, debug=False)
    qc = nc.dram_tensor("qc", [PAIRS_PER_CORE, 128, S], F32, kind="ExternalInput")
    kc = nc.dram_tensor("kc", [PAIRS_PER_CORE, 128, S], F32, kind="ExternalInput")
    out = nc.dram_tensor("out", [PAIRS_PER_CORE, S, S], F32, kind="ExternalOutput")

    HC = S // 2  # half-row chunk for input DMA / sin / evac / out DMA

    with tile.TileContext(nc) as tc:
        with (
            tc.tile_pool(name="raw", bufs=2) as rawpool,
            tc.tile_pool(name="uv", bufs=2) as uvpool,
            tc.tile_pool(name="ot", bufs=8) as opool,
            tc.tile_pool(name="psum", bufs=2, space="PSUM") as ppool,
        ):
            for p in range(PAIRS_PER_CORE):
                # Pair 0 inputs ride both HWDGE queues (empty at t=0, fast
                # head); later pairs ride the gpsimd SWDGE queue so they
                # never delay output DMAs on the SP/ACT hardware queues.
                qraw = rawpool.tile([128, S], F32, tag="qraw")
                kraw = rawpool.tile([128, S], F32, tag="kraw")
                u = uvpool.tile([128, S], F32R, tag="u")
                v = uvpool.tile([128, S], F32R, tag="v")
                for h in range(2):
                    hs = slice(h * HC, (h + 1) * HC)
                    if p == 0:
                        eng = nc.sync if h == 0 else nc.scalar
                        eng.dma_start(out=kraw[:, hs], in_=kc[p, :, hs])
                        eng.dma_start(out=qraw[:, hs], in_=qc[p, :, hs])
                    else:
                        nc.gpsimd.dma_start(out=kraw[:, hs], in_=kc[p, :, hs])
                        nc.gpsimd.dma_start(out=qraw[:, hs], in_=qc[p, :, hs])
                for h in range(2):
                    hs = slice(h * HC, (h + 1) * HC)
                    nc.scalar.activation(
                        v[:, hs], kraw[:, hs], mybir.ActivationFunctionType.Sin
                    )
                for h in range(2):
                    hs = slice(h * HC, (h + 1) * HC)
                    nc.scalar.activation(
                        u[:, hs], qraw[:, hs], mybir.ActivationFunctionType.Sin
                    )

                for q in range(N_QT):
                    ps = ppool.tile([128, N_KT * K_TILE], F32, tag="ps")
                    for k in range(N_KT):
                        nc.tensor.matmul(
                            ps[:, k * K_TILE : (k + 1) * K_TILE],
                            u[:, q * Q_TILE : (q + 1) * Q_TILE],
                            v[:, k * K_TILE : (k + 1) * K_TILE],
                            start=True,
                            stop=True,
                        )
                    ot = opool.tile([128, S], F32, tag="ot")
                    for j in range(2):
                        hs = slice(j * HC, (j + 1) * HC)
                        # Evac PSUM->SBUF with the 1/64 scale, alternating
                        # engines; the out-DMA goes crosswise so each HWDGE
                        # queue (SP / ACT) carries half the output traffic.
                        if (2 * q + j) % 2 == 0:
                            nc.vector.tensor_scalar_mul(ot[:, hs], ps[:, hs], 1.0 / D)
                            nc.scalar.dma_start(
                                out=out[p, q * Q_TILE : (q + 1) * Q_TILE, hs],
                                in_=ot[:, hs],
                            )
                        else:
                            nc.scalar.mul(ot[:, hs], ps[:, hs], 1.0 / D)
                            nc.sync.dma_start(
                                out=out[p, q * Q_TILE : (q + 1) * Q_TILE, hs],
                                in_=ot[:, hs],
                            )
    nc.compile()
    return nc


def _prep(ph):
    """[16, S, D] phases -> [16, 128, S] stacked cos/sin arguments."""
    pht = ph.astype(np.float64).transpose(0, 2, 1)  # [16, D, S]
    cos_arg = np.mod(pht + (np.pi / 2 + np.pi), 2 * np.pi) - np.pi
    sin_arg = np.mod(pht + np.pi, 2 * np.pi) - np.pi
    return np.concatenate([cos_arg, sin_arg], axis=1).astype(np.float32)


def kernel(phases_q, phases_k, _trace=False):
    pq = np.asarray(phases_q, dtype=np.float32).reshape(B * H, S, D)
    pk = np.asarray(phases_k, dtype=np.float32).reshape(B * H, S, D)
    qc = _prep(pq)  # [16, 128, S]
    kc = _prep(pk)

    in_maps = []
    for c in range(N_CORES):
        sl = slice(c * PAIRS_PER_CORE, (c + 1) * PAIRS_PER_CORE)
        in_maps.append(
            {"qc": np.ascontiguousarray(qc[sl]), "kc": np.ascontiguousarray(kc[sl])}
        )

    if "nc" not in _NC_CACHE:
        _NC_CACHE["nc"] = build_kernel()
    nc = _NC_CACHE["nc"]

    res = run_bass_kernel_spmd(
        nc, in_maps, core_ids=list(range(N_CORES)), trace=_trace
    )
    full = np.concatenate([r["out"] for r in res.results], axis=0)
    out = full.reshape(B, H, S, S)
    if _trace:
        return out, res
    return out
